# revision 6
# baseline (speedup 1.0000x reference)
# Trainium2 Bass kernel for BertNER head — transpose-gather, dest-major.
#   out = softmax(compact_valid(x) @ W + b)
#
# compact_valid moves each row's valid tokens to its prefix, so the only
# rows of X that influence the output are the valid ones (~50%).  The
# host computes, per 128-token destination chunk, the source X row index
# of each slot; X is host-cast to bf16 once and the device reads ONLY
# the valid rows (~4.2MB/core instead of 16MB) with one
# dma_gather(transpose=True) per batch row, which also lands X^T
# (h on partitions) directly in SBUF — no PE transposes, no PSUM X
# drains.  Trailing pad slots use index -1 with the true per-row valid
# count passed via a Pool register (reg_load from a per-core input), so
# pads generate no descriptors and no HBM traffic.  Gathers run on TWO
# SWDGE queues split by contiguous halves (rows 0-3 -> q0, rows 4-7 ->
# q1): each queue's index stream stays ascending in a disjoint address
# region, so DRAM page locality survives while transfers parallelize
# (paired A/B: 1.8x over one queue; a fine-grained 4-queue rotation
# measured ~2x slower).  Small gather calls matter too: 384-descriptor
# calls sustain ~2-3x the per-descriptor rate of 768-desc calls.
#
# Logits zT = W^T @ X^T (8 bf16 matmuls of K*128 columns per row, W
# host-permuted to plain h-chunk order), small PE transpose back to
# [tok, 9], then a bitwise-AND mask drain (pad lanes hold stale SBUF
# data, possibly NaN — AND with 0 kills them where a multiply would
# propagate NaN) so pad logits become 0 and softmax(0 + b) = softmax(b),
# the reference's padded-tail value.  Bias + softmax on ACT/DVE (logits
# are tiny: |z| < ~5, so no max-subtraction), dense 36B-run out DMA.
# Dest chunks wholly beyond any core's valid prefix get a softmax(b)
# const fill.
#
# Sharding: pure data parallel over batch, 8 rows per core.
import numpy as np
import ml_dtypes

B, S, H, L = 64, 512, 1024, 9
NCORES = 8
BL = B // NCORES      # batch rows per core
T = BL * S            # tokens per core
P = 128
HC = H // P           # 8 h-chunks
NSC = S // P          # max dest chunks per row (4)

_cache = {}


def _build(Ks, reps=1, mode="full"):
    import concourse.bass as bass
    import concourse.mybir as mybir
    import concourse.tile as tile
    from concourse import bacc

    f32 = mybir.dt.float32
    bf16 = mybir.dt.bfloat16
    i16 = mybir.dt.int16
    i32 = mybir.dt.int32

    NCtot = sum(Ks)
    offs = np.concatenate([[0], np.cumsum(Ks)]).astype(int)

    nc = bacc.Bacc(
        "TRN2",
        target_bir_lowering=False,
        debug=False,
        enable_asserts=False,
        num_devices=NCORES,
        num_swdge_queues=4,
    )

    x = nc.dram_tensor("x", (T, H), bf16, kind="ExternalInput").ap()
    w = nc.dram_tensor("w", (P, HC, L), bf16, kind="ExternalInput").ap()
    bbs = nc.dram_tensor("bbs", (P, 2, L), f32, kind="ExternalInput").ap()
    gidx = nc.dram_tensor("gidx", (P, max(NCtot * 8, 1)), i16, kind="ExternalInput").ap()
    vflag = nc.dram_tensor("vflag", (P, max(NCtot, 1)), i32, kind="ExternalInput").ap()
    nvals = nc.dram_tensor("nvals", (1, BL), i32, kind="ExternalInput").ap()
    idf = nc.dram_tensor("id_f32", (P, P), f32, kind="ExternalInput").ap()
    out = nc.dram_tensor("out", (T, L), f32, kind="ExternalOutput").ap()
    outg = out.rearrange("(g s) l -> g s l", s=S)

    AL = mybir.AluOpType
    AF = mybir.ActivationFunctionType

    # contiguous runs of equal K for the const-fill batching
    runs = []
    g0 = 0
    for g in range(1, BL + 1):
        if g == BL or Ks[g] != Ks[g0]:
            runs.append((g0, g, Ks[g0]))
            g0 = g

    with tile.TileContext(nc) as tc:
        with (
            tc.tile_pool(name="consts", bufs=1) as cpool,
            tc.tile_pool(name="xin", bufs=5) as xpool,
            tc.tile_pool(name="z", bufs=3) as zpool,
            tc.tile_pool(name="small", bufs=3) as spool,
            tc.tile_pool(name="outp", bufs=3) as opool,
            tc.tile_pool(name="psacc", bufs=3, space="PSUM") as psacc,
            tc.tile_pool(name="psz", bufs=3, space="PSUM") as psz,
        ):
            # ---- constants (parallel queues; gidx first: gathers need it) ----
            gidx_sb = cpool.tile([P, max(NCtot * 8, 1)], i16)
            nc.sync.dma_start(gidx_sb, gidx)
            vflag_sb = cpool.tile([P, max(NCtot, 1)], i32)
            nc.scalar.dma_start(vflag_sb, vflag)
            nvals_sb = cpool.tile([1, BL], i32)
            nc.sync.dma_start(nvals_sb, nvals)
            nv_regs = []
            for g in range(BL):
                r = nc.gpsimd.alloc_register(f"nv{g}")
                nc.gpsimd.reg_load(r, nvals_sb[0:1, g : g + 1])
                nv_regs.append(r)
            id_f = cpool.tile([P, P], f32)
            nc.scalar.dma_start(id_f, idf)
            bbs_sb = cpool.tile([P, 2, L], f32)
            nc.scalar.dma_start(bbs_sb, bbs)
            bb_sb = bbs_sb[:, 0, :]
            softb_sb = bbs_sb[:, 1, :]
            softbig = cpool.tile([P, BL, NSC, L], f32)
            nc.vector.tensor_copy(
                out=softbig,
                in_=softb_sb[:, None, None, :].to_broadcast((P, BL, NSC, L)),
            )
            # W in plain h order: chunk hc holds rows h = hc*128 + k
            w_sb = cpool.tile([P, HC, L], bf16)
            nc.sync.dma_start(w_sb, w)

            for _rep in range(reps):
                # per-row gathers: X^T lands in SBUF
                xgs = {}
                for g in range(BL):
                    K = Ks[g]
                    if K == 0:
                        continue
                    ni = K * P
                    xg = xpool.tile([P, HC, ni], bf16, name="xg", tag="xg")
                    if mode == "nogather":
                        nc.vector.memset(xg[:, 0, 0:2], 0)
                    else:
                        nc.gpsimd.dma_gather(
                            xg,
                            x,
                            gidx_sb[:, offs[g] * 8 : offs[g] * 8 + ni // 16],
                            ni,
                            nv_regs[g],
                            H,
                            transpose=True,
                            queue_num=0 if g < BL // 2 else 1,
                        )
                    xgs[g] = xg

                # const softmax(b) fill for tails beyond the valid prefix.
                # DMA APs balance at most 3 dims, so a multi-row run with a
                # multi-chunk tail must be emitted per row.
                for r0, r1, K in runs:
                    if K >= NSC:
                        continue
                    if r1 - r0 > 1 and NSC - K > 1:
                        for rr in range(r0, r1):
                            nc.sync.dma_start(
                                outg[rr, K * P :, :].rearrange(
                                    "(j p) l -> p j l", p=P
                                ),
                                softbig[:, rr, : NSC - K, :],
                            )
                    else:
                        nc.sync.dma_start(
                            outg[r0:r1, K * P :, :].rearrange(
                                "g (j p) l -> p g j l", p=P
                            ),
                            softbig[:, r0:r1, : NSC - K, :],
                        )

                for g in range(BL):
                    K = Ks[g]
                    if K == 0 or mode == "gatheronly":
                        continue
                    off = int(offs[g])
                    xg = xgs[g]

                    # zT = W^T @ X^T for the whole row: 8 K*128-col streams
                    zTp = psacc.tile([L, K, P], f32, name="zTp", tag="acc")
                    for hc in range(HC):
                        nc.tensor.matmul(
                            zTp,
                            w_sb[:, hc, :],
                            xg[:, hc, :],
                            start=(hc == 0),
                            stop=(hc == HC - 1),
                        )
                    zTs = zpool.tile([L, K, P], f32, name="zTs", tag="zTs")
                    if g % 2 == 0:
                        nc.scalar.copy(out=zTs, in_=zTp)
                    else:
                        nc.vector.tensor_copy(out=zTs, in_=zTp)

                    # back to [tok, 9]; flag-mult drain kills pad logits
                    zf_row = psz.tile([P, K, L], f32, name="zf_row", tag="zf")
                    for k in range(K):
                        nc.tensor.matmul(
                            zf_row[:, k, :],
                            zTs[:, k, :],
                            id_f[:L, :L],
                            is_transpose=True,
                            start=True,
                            stop=True,
                        )
                    cb = spool.tile([P, K, L], f32, name="cb", tag="cb")
                    nc.vector.tensor_tensor(
                        out=cb.bitcast(i32),
                        in0=zf_row.bitcast(i32),
                        in1=vflag_sb[:, off : off + K, None].to_broadcast((P, K, L)),
                        op=AL.bitwise_and,
                    )

                    # softmax over the last dim (9)
                    eb = spool.tile([P, K, L], f32, name="eb", tag="eb")
                    nc.vector.tensor_tensor(
                        out=eb,
                        in0=cb,
                        in1=bb_sb[:, None, :].to_broadcast((P, K, L)),
                        op=AL.add,
                    )
                    e_t = spool.tile([P, K, L], f32, name="e_t", tag="e")
                    nc.scalar.activation(e_t, eb, AF.Exp)
                    es = spool.tile([P, K], f32, name="es", tag="es")
                    nc.vector.reduce_sum(es, e_t, axis=mybir.AxisListType.X)
                    ri = spool.tile([P, K], f32, name="ri", tag="ri")
                    nc.vector.reciprocal(ri, es)
                    outt = opool.tile([P, K, L], f32, name="outt", tag="outt")
                    nc.vector.tensor_tensor(
                        out=outt,
                        in0=e_t,
                        in1=ri[:, :, None].to_broadcast((P, K, L)),
                        op=AL.mult,
                    )
                    nc.sync.dma_start(
                        outg[g, : K * P, :].rearrange("(j p) l -> p j l", p=P),
                        outt,
                    )

    nc.compile()
    return nc


def _get_nc(Ks):
    key = tuple(Ks)
    if key not in _cache:
        _cache[key] = _build(key)
    return _cache[key]


def _plan(valid_counts):
    vc = valid_counts.reshape(NCORES, BL)
    return tuple(int(k) for k in np.ceil(vc / P).max(axis=0).astype(int))


def _wrap16(idx):
    # dma_gather idx layout: index i -> partition 16*g + i%16 (all 8 groups
    # replicated), column i//16
    n = len(idx)
    t = np.empty((P, n // 16), dtype=np.int16)
    cols = idx.reshape(n // 16, 16).T
    for g in range(8):
        t[16 * g : 16 * g + 16, :] = cols
    return t


def _make_in_maps(sequence_output, valid_mask, W, b, Ks):
    xs = np.asarray(sequence_output)
    mk = np.ascontiguousarray(np.asarray(valid_mask), dtype=np.int32)
    Wf = np.asarray(W, dtype=np.float32)
    bf = np.asarray(b, dtype=np.float32)

    NCtot = max(sum(Ks), 1)
    offs = np.concatenate([[0], np.cumsum(Ks)]).astype(int)

    x_bf = np.ascontiguousarray(xs.reshape(B, S, H).astype(ml_dtypes.bfloat16))
    # W in plain h order, host-cast: [k, hc, l] holds W[hc*128 + k, l]
    w_bf = np.ascontiguousarray(
        Wf.reshape(HC, P, L).transpose(1, 0, 2).astype(ml_dtypes.bfloat16)
    )

    idf_np = np.eye(P, dtype=np.float32)
    eb = np.exp(bf - bf.max())
    softb = (eb / eb.sum()).astype(np.float32)
    bbs_np = np.ascontiguousarray(
        np.broadcast_to(np.stack([bf, softb]), (P, 2, L)).astype(np.float32)
    )

    in_maps = []
    for c in range(NCORES):
        gidx_np = np.full((P, NCtot * 8), -1, dtype=np.int16)
        vflag_np = np.zeros((P, NCtot), dtype=np.int32)
        nvals_np = np.zeros((1, BL), dtype=np.int32)
        for g in range(BL):
            (vi,) = np.nonzero(mk[c * BL + g])
            nv = len(vi)
            K = Ks[g]
            if K == 0:
                continue
            pad_len = K * P - nv
            pos = np.concatenate([g * S + vi, np.full(pad_len, -1, dtype=np.int64)])
            gidx_np[:, offs[g] * 8 : (offs[g] + K) * 8] = _wrap16(pos)
            vflag_np[:, offs[g] : offs[g] + K] = np.where(
                np.arange(K * P).reshape(K, P).T < nv, -1, 0
            )
            nvals_np[0, g] = nv
        in_maps.append(
            {
                "x": x_bf[c * BL : (c + 1) * BL].reshape(T, H),
                "w": w_bf,
                "bbs": bbs_np,
                "gidx": gidx_np,
                "vflag": vflag_np,
                "nvals": nvals_np,
                "id_f32": idf_np,
            }
        )
    return in_maps


def kernel(sequence_output, valid_mask, W, b):
    from concourse.bass_utils import run_bass_kernel_spmd

    mk = np.asarray(valid_mask)
    Ks = _plan(mk.sum(axis=1))
    nc = _get_nc(Ks)
    in_maps = _make_in_maps(sequence_output, valid_mask, W, b, Ks)
    res = run_bass_kernel_spmd(nc, in_maps, core_ids=list(range(NCORES)))
    _cache["last_results"] = res

    outs = [res.results[c]["out"].reshape(BL, S, L) for c in range(NCORES)]
    return np.concatenate(outs, axis=0).astype(np.float32)
